# revision 20
# baseline (speedup 1.0000x reference)
"""Trainium2 Bass kernel for the NeuralSDE Q-model scan.

Strategy
--------
Data-parallel over paths: 65536 paths -> 8 cores x 8192 paths, no
cross-core communication. Per core the 8192 paths live as one
(128 partitions x 64 free) tile.

The per-step MLP+tanh (lambda = 3*tanh(MLP(log_v, t))) is replaced by a
per-step degree-D polynomial in the scaled state xi = (log_v + 2.5)/4.5,
fit on host from the (input-provided) MLP weights over the full clipped
range log_v in [-7, 2] (fit max-err ~1e-4 at D=11, ~1e-5 at D=14).

The carried state is xi itself (in [-1, 1]); all affine constants fold
into the polynomial chain / fused-op scalars.

Phase A (sequential 512-step recurrence; one custom-DVE chain per step):
    r   = T_t(xi)                 H3INIT + k x HORNER3   (D = 2+3k)
    v   = alpha*xi + c + r        AFFINE_THEN_ADD
    xi' = clip((s1/h)*z1_t + v)   CLIPADD (bounds -1/1)
    Q  += (c*r)^2                 SQACC
xi' is written into an SBUF-resident history stack (128, 64, 513);
z1 arrives step-major (host-transposed input) so each step reads one
contiguous 32KB row.

Phase B (bulk, per path-block b of 128 paths; steps contiguous):
    x    = h*xi + m               tensor_scalar      -> log_v output rows
    vol  = |c2|*exp(x/2)          ACT Exp (scale h/2, bias m/2+ln|c2|)
    ls   = cumsum_t(R*dt - dt/2*e^x - vol*zs)   LSCAN (fused scan)
    spot = exp(ls)                ACT Exp
where zs = z1 + (c3/c2)*z2 is combined on host (input marshaling).

Outputs stream path-major (contiguous 2KB rows per partition).
"""

import os
import sys

if "/opt/trn_rl_repo" not in sys.path:
    sys.path.insert(0, "/opt/trn_rl_repo")

import numpy as np

import concourse.bass as bass
import concourse.mybir as mybir
import concourse.dve_ops as dve_ops
from concourse import tile
from concourse.bass_utils import run_bass_kernel_spmd
from concourse.dve_spec import (
    Spec, Src0, Src1, C0, C1, C2, AluOp, lower, maxx, minn, sq, scan,
    _has_src1,
)
from concourse.dve_uop import DveOpSpec

Alu = mybir.AluOpType
Act = mybir.ActivationFunctionType
F32 = mybir.dt.float32

# ---- problem constants (hardcoded; must match the reference) ----
N_PATHS = 65536
N_STEPS = 512
DT = 1.0 / 252.0
KAPPA = 2.72
THETA = -3.5
SIGMA_P = (0.1 + 1.6) / 2.0
RHO = -0.85
R = 0.0373
LAMBDA_MAX = 3.0
LOG_V_MIN = -7.0
LOG_V_MAX = 2.0

N_CORES = 8
P_LOCAL = N_PATHS // N_CORES          # 8192
NQ = 128                              # partitions
NF = P_LOCAL // NQ                    # 64 path-blocks (free dim)

# polynomial degree D = 2 + 3k
DEGREE = int(os.environ.get("NSDE_DEGREE", "11"))
PHASES = os.environ.get("NSDE_PHASES", "ab")
BCHUNKS = int(os.environ.get("NSDE_BCHUNKS", "1"))
assert DEGREE % 3 == 2, "DEGREE must be 2 mod 3 (H3INIT + k*HORNER3)"
XI_M = -2.5                           # x = XI_M + XI_H * xi
XI_H = 4.5

ALPHA = 1.0 - KAPPA * DT
BETA = KAPPA * THETA * DT
S1 = SIGMA_P * np.sqrt(DT)            # noise scale on log_v
CA = LAMBDA_MAX * SIGMA_P * DT        # p~ = -CA * tanh(raw)
SQ_SCALE = 1.0 / (SIGMA_P * np.sqrt(DT))   # (SQ_SCALE*p~)^2 == lambda^2*dt
C2C = RHO * np.sqrt(DT)               # < 0
C3C = np.sqrt(1.0 - RHO * RHO) * np.sqrt(DT)
C32 = C3C / C2C
SQV_SCALE = float(np.sqrt(DT / 2.0) / abs(C2C))  # (SQV_SCALE*vol_s)^2 = dt/2*e^x
RDT = R * DT
# phase-A folded constants (xi-space)
B1_SCALE = ALPHA                                   # v = alpha*xi + B1_BIAS + r/h
B1_BIAS = (ALPHA * XI_M + BETA - XI_M) / XI_H
Z_SCALE = S1 / XI_H
VOL_SCALE = XI_H / 2.0
VOL_BIAS = float(XI_M / 2.0 + np.log(abs(C2C)))

MAX_WAITS = 1

# ---------------------------------------------------------------------------
# custom DVE ops (registered at import; appended to dve_ops.OPS)
# ---------------------------------------------------------------------------

def _register_dve_op(name, spec):
    for op in dve_ops.OPS:
        if op.name == name:
            return op
    row = dve_ops._CUSTOM_DVE_ROW_BASE + len(dve_ops.OPS)
    assert row < 0x20
    dve_ops._SUB_OPCODE_FOR_NAME[name] = row
    shas = {}
    for ver in ("v3", "v4"):
        s = DveOpSpec(name=name, opcode=row, uops=lower(spec, ver=ver),
                      rd1_en=_has_src1(spec))
        shas[ver] = s.sha(ver)
    op = dve_ops.DveOp(name, spec, subdim=False, uops_sha=shas)
    dve_ops.OPS.append(op)
    dve_ops.CUSTOM_DVE_SPECS[name] = spec
    return op


# r = (x*c0 + c1)*x + c2            (degree-2 seed, single tensor stream)
H3INIT = _register_dve_op("NSDE_H3INIT", Spec(
    body=(Src0 * C0 + C1) * Src0 + C2,
    reference=lambda in0, in1, s0, s1, imm2:
        ((in0.astype(np.float32) * np.float32(s0) + np.float32(s1))
         * in0 + np.float32(imm2)).astype(np.float32),
))

# r' = ((r*x + c0)*x + c1)*x + c2   (three Horner steps)
HORNER3 = _register_dve_op("NSDE_HORNER3", Spec(
    body=((Src1 * Src0 + C0) * Src0 + C1) * Src0 + C2,
    reference=lambda in0, in1, s0, s1, imm2:
        (((in1.astype(np.float32) * in0 + np.float32(s0)) * in0
          + np.float32(s1)) * in0 + np.float32(imm2)).astype(np.float32),
))

# out = clip(z*c0 + v, c1, c2)
CLIPADD = _register_dve_op("NSDE_CLIPADD", Spec(
    body=minn(maxx(Src0 * C0 + Src1, C1), C2),
    reference=lambda in0, in1, s0, s1, imm2:
        np.minimum(np.maximum(in0.astype(np.float32) * np.float32(s0) + in1,
                              np.float32(s1)), np.float32(imm2)).astype(np.float32),
))

# Q' = Q + (r*c0)^2
SQACC = _register_dve_op("NSDE_SQACC", Spec(
    body=Src1 + sq(Src0 * C0),
    reference=lambda in0, in1, s0, s1, imm2:
        (in1 + (in0.astype(np.float32) * np.float32(s0)) ** 2).astype(np.float32),
))

# y = ((v - xi*c0 - c1) * c2)^2   (extract p~ from folded v, square)
QSQ = _register_dve_op("NSDE_QSQ", Spec(
    body=sq((Src0 - Src1 * C0 - C1) * C2),
    reference=lambda in0, in1, s0, s1, imm2:
        (((in0.astype(np.float32) - in1 * np.float32(s0) - np.float32(s1))
          * np.float32(imm2)) ** 2).astype(np.float32),
))

# ls = init(c0) + cumsum((imm2 - (vol*c1)^2) - vol*zs)
LSCAN = _register_dve_op("NSDE_LSCAN2", Spec(
    body=scan(AluOp.ADD, (C2 - sq(Src0 * C1)) - Src0 * Src1, init=C0),
    reference=lambda in0, in1, s0, s1, imm2:
        (np.asarray(s0, np.float32).reshape(-1, 1)
         + np.cumsum((np.float32(imm2)
                      - (in0.astype(np.float32) * np.float32(s1)) ** 2)
                     - in0 * in1, axis=-1, dtype=np.float32)).astype(np.float32),
))


def _gelu_tanh(x):
    return 0.5 * x * (1.0 + np.tanh(np.sqrt(2 / np.pi) * (x + 0.044715 * x ** 3)))


def _fit_polys(W1, b1, W2, b2, W3, b3, n_steps, degree):
    """Per-step poly coefficients of p~(xi) ~= -CA*tanh(raw(x, t)) in the
    xi power basis; returns (n_steps, degree+1) float64."""
    W1 = W1.astype(np.float64); b1 = b1.astype(np.float64)
    W2 = W2.astype(np.float64); b2 = b2.astype(np.float64)
    W3 = W3.astype(np.float64); b3 = b3.astype(np.float64)
    t_idx = np.arange(n_steps, dtype=np.float64) * DT / (n_steps * DT)

    G = max(4 * (degree + 1), 96)
    k = np.arange(G)
    xg = np.cos(np.pi * (k + 0.5) / G)            # xi nodes in [-1, 1]
    x = XI_M + XI_H * xg

    X = np.stack([np.repeat(x[None, :], n_steps, 0),
                  np.repeat(t_idx[:, None], G, 1)], axis=-1)
    h = _gelu_tanh(X @ W1.T + b1)
    h = _gelu_tanh(h @ W2.T + b2)
    raw = (h @ W3.T + b3)[..., 0]
    f = -CA * np.tanh(raw)

    V = np.polynomial.chebyshev.chebvander(xg, degree)
    coef, *_ = np.linalg.lstsq(V, f.T, rcond=None)
    gam = np.empty((n_steps, degree + 1), np.float64)
    for t in range(n_steps):
        gam[t] = np.polynomial.chebyshev.cheb2poly(coef[:, t])
    return gam


def _split_waits(nc):
    """This walrus build allows only one sync-wait per instruction; push
    extra waits onto preceding same-engine NoOps."""
    for bb in nc.main_func.blocks:
        newlist = []
        for ins in bb.instructions:
            si = ins.sync_info
            if si is not None and si.on_wait and len(si.on_wait) > MAX_WAITS:
                waits = list(si.on_wait)
                extra, keep = waits[:-MAX_WAITS], waits[-MAX_WAITS:]
                for i, w in enumerate(extra):
                    nop = mybir.InstNoOp(
                        name=f"{ins.name}-ws{i}", engine=ins.engine,
                        ins=[], outs=[],
                        sync_info=mybir.SyncInfo(on_wait=[w], on_update=[]))
                    nc.register_instruction(nop, overwrite=True)
                    newlist.append(nop)
                ins.sync_info = mybir.SyncInfo(on_wait=keep,
                                               on_update=list(si.on_update))
            newlist.append(ins)
        bb.instructions[:] = newlist


def chain_consts(gam_t, fold_affine=True):
    """Map poly coeffs (in xi, p~ units) to the H3INIT/HORNER3 constants.
    With fold_affine the chain directly produces
    v = alpha*xi + B1_BIAS + T(xi)/XI_H."""
    T = np.asarray(gam_t, np.float64) / XI_H
    if fold_affine:
        T = T.copy()
        T[1] += ALPHA
        T[0] += B1_BIAS
    D = len(T) - 1
    k = (D - 2) // 3
    init = (float(T[D]), float(T[D - 1]), float(T[D - 2]))
    steps = []
    for j in range(1, k + 1):
        base = 3 * (k - j)
        steps.append((float(T[base + 2]), float(T[base + 1]), float(T[base])))
    return init, steps


def build_program(gam, init_log_v, n_steps):
    """Emit the bass program. gam: (n_steps, D+1) float64, p~ units."""
    nc = bass.Bass()

    zsm = nc.dram_tensor("zsm", [P_LOCAL, N_STEPS], F32, kind="ExternalInput")
    z1t = nc.dram_tensor("z1t", [N_STEPS, P_LOCAL], F32, kind="ExternalInput")
    lv_out = nc.dram_tensor("lv", [P_LOCAL, N_STEPS], F32, kind="ExternalOutput")
    sp_out = nc.dram_tensor("sp", [P_LOCAL, N_STEPS], F32, kind="ExternalOutput")
    q_out = nc.dram_tensor("lsq", [P_LOCAL], F32, kind="ExternalOutput")

    zs_v = zsm.rearrange("(q f) s -> q f s", q=NQ)    # [128, 64, 512]
    lv_v = lv_out.rearrange("(q f) s -> q f s", q=NQ)
    sp_v = sp_out.rearrange("(q f) s -> q f s", q=NQ)
    z1t_v = z1t.rearrange("s (q f) -> s q f", q=NQ)   # [512, 128, 64]
    q_v = q_out.rearrange("(q f) -> q f", q=NQ)       # [128, 64]

    xi0 = (float(init_log_v) - XI_M) / XI_H

    with tile.TileContext(nc) as tc:
        with (
            tc.tile_pool(name="stack", bufs=1) as stack_pool,
            tc.tile_pool(name="state", bufs=1) as state_pool,
            tc.tile_pool(name="work", bufs=2) as work,
            tc.tile_pool(name="zin", bufs=4) as zin,
            tc.tile_pool(name="bulk", bufs=3) as bulk,
            tc.tile_pool(name="bout", bufs=3) as bout,
            tc.tile_pool(name="consts", bufs=1) as consts,
        ):
            stack = stack_pool.tile([NQ, NF, n_steps + 1], F32, tag="stack")
            qacc = state_pool.tile([NQ, NF], F32, tag="qacc")
            QW = min(64, n_steps)
            qring = state_pool.tile([NQ, NF, QW], F32, tag="qring")
            zerocol = consts.tile([NQ, 1], F32, tag="zerocol")
            volb = consts.tile([NQ, 1], F32, tag="volb")

            nc.vector.memset(zerocol[:], 0.0)
            nc.vector.memset(volb[:], VOL_BIAS)
            nc.vector.memset(qacc[:], 0.0)
            nc.vector.memset(stack[:, :, 0], xi0)

            # ---------------- Phase A: sequential scan ----------------
            for t in range(n_steps):
                (ia, ib, ic), hsteps = chain_consts(gam[t])
                xt = stack[:, :, t]
                z1tile = zin.tile([NQ, NF], F32, tag="z1tile")
                nc.sync.dma_start(z1tile[:], z1t_v[t])

                # v = alpha*xi + B1_BIAS + p~(xi)/h  (affine folded into chain)
                v = work.tile([NQ, NF], F32, tag="v")
                nc.vector._custom_dve(H3INIT, out=v[:], in0=xt,
                                      s0=ia, s1=ib, imm2=ic)
                for (d0, d1, d2) in hsteps:
                    nc.vector._custom_dve(HORNER3, out=v[:], in0=xt, in1=v[:],
                                          s0=d0, s1=d1, imm2=d2)
                nc.vector._custom_dve(CLIPADD, out=stack[:, :, t + 1],
                                      in0=z1tile[:], in1=v[:],
                                      s0=float(Z_SCALE), s1=-1.0, imm2=1.0)
                # lambda^2*dt into the Q ring; summed per window below
                nc.vector._custom_dve(QSQ, out=qring[:, :, t % QW],
                                      in0=v[:], in1=xt,
                                      s0=float(ALPHA), s1=float(B1_BIAS),
                                      imm2=float(SQ_SCALE * XI_H))
                if (t + 1) % QW == 0 or t == n_steps - 1:
                    w = (t % QW) + 1
                    qpart = work.tile([NQ, NF], F32, tag="qpart")
                    nc.vector.tensor_reduce(qpart[:], qring[:, :, 0:w],
                                            mybir.AxisListType.X, Alu.add)
                    nc.vector.scalar_tensor_tensor(
                        qacc[:], qpart[:], 1.0, qacc[:], Alu.mult, Alu.add)

            nc.sync.dma_start(q_v[:, :], qacc[:])

            # ---------------- Phase B: bulk per path-block ----------------
            tails = consts.tile([NQ, NF], F32, tag="tails")
            for c in range(BCHUNKS if "b" in PHASES else 0):
                lo = c * n_steps // BCHUNKS
                hi = (c + 1) * n_steps // BCHUNKS
                L = hi - lo
                for b in range(NF):
                    xs_pre = stack[:, b, lo:hi]       # xi before each step
                    zsb = bulk.tile([NQ, L], F32, tag="zsb")
                    nc.sync.dma_start(zsb[:], zs_v[:, b, lo:hi])

                    xo = bout.tile([NQ, L], F32, tag="xo")
                    nc.gpsimd.tensor_scalar(xo[:], stack[:, b, lo + 1:hi + 1],
                                            XI_H, XI_M, Alu.mult, Alu.add)
                    nc.sync.dma_start(lv_v[:, b, lo:hi], xo[:])

                    vol = bulk.tile([NQ, L], F32, tag="vol")
                    nc.scalar.activation(vol[:], xs_pre, Act.Exp,
                                         bias=volb[:], scale=float(VOL_SCALE))
                    ls = bulk.tile([NQ, L], F32, tag="ls")
                    init = 0.0 if c == 0 else tails[:, b:b + 1]
                    nc.vector._custom_dve(LSCAN, out=ls[:], in0=vol[:],
                                          in1=zsb[:], s0=init,
                                          s1=float(SQV_SCALE), imm2=float(RDT))
                    if c < BCHUNKS - 1:
                        nc.vector.tensor_scalar_add(tails[:, b:b + 1],
                                                    ls[:, L - 1:L], 0.0)
                    spot = bout.tile([NQ, L], F32, tag="spot")
                    nc.scalar.activation(spot[:], ls[:], Act.Exp,
                                         bias=zerocol[:], scale=1.0)
                    nc.sync.dma_start(sp_v[:, b, lo:hi], spot[:])

    # Populate .instr bytes for InstCustomDveAnt (raw Bass skips this pass;
    # without it walrus fails with "ISA wrong length").
    mybir.codegen_inst_isa_subclasses(nc)
    _split_waits(nc)
    return nc


def model_numpy(gam, init_log_v, z1s, zss, ns):
    """fp32 numpy model of exactly the device math (for bring-up tests)."""
    xi = np.full((z1s.shape[0],), np.float32((init_log_v - XI_M) / XI_H))
    Q = np.zeros_like(xi)
    ls = np.zeros_like(xi)
    lv = np.zeros((z1s.shape[0], ns), np.float32)
    sp = np.zeros((z1s.shape[0], ns), np.float32)
    QW = min(64, ns)
    qwin = []
    for t in range(ns):
        (ia, ib, ic), hsteps = chain_consts(gam[t])
        v = (np.float32(ia) * xi + np.float32(ib)) * xi + np.float32(ic)
        for (d0, d1, d2) in hsteps:
            v = ((v * xi + np.float32(d0)) * xi + np.float32(d1)) * xi + np.float32(d2)
        xin = np.minimum(np.maximum(
            np.float32(Z_SCALE) * z1s[:, t] + v, np.float32(-1.0)), np.float32(1.0))
        y = ((v - np.float32(ALPHA) * xi - np.float32(B1_BIAS))
             * np.float32(SQ_SCALE * XI_H)) ** 2
        qwin.append(y)
        if (t + 1) % QW == 0 or t == ns - 1:
            s = qwin[0]
            for yy in qwin[1:]:
                s = s + yy
            Q = Q + s
            qwin = []
        vol = np.exp(np.float32(VOL_SCALE) * xi + np.float32(VOL_BIAS))
        dls = (np.float32(RDT) - (np.float32(SQV_SCALE) * vol) ** 2) - vol * zss[:, t]
        ls = ls + dls
        lv[:, t] = np.float32(XI_H) * xin + np.float32(XI_M)
        sp[:, t] = np.exp(ls)
        xi = xin
    return lv, sp, Q


def kernel(z1, z2, W1, b1, W2, b2, W3, b3, init_log_v):
    z1 = np.ascontiguousarray(np.asarray(z1, np.float32))
    z2 = np.asarray(z2, np.float32)
    zs = np.ascontiguousarray(z1 + np.float32(C32) * z2)
    gam = _fit_polys(np.asarray(W1), np.asarray(b1), np.asarray(W2),
                     np.asarray(b2), np.asarray(W3), np.asarray(b3),
                     N_STEPS, DEGREE)

    nc = build_program(gam, float(np.asarray(init_log_v)), N_STEPS)

    in_maps = []
    for c in range(N_CORES):
        sl = slice(c * P_LOCAL, (c + 1) * P_LOCAL)
        z1s = z1[sl]
        in_maps.append({
            "zsm": zs[sl],
            "z1t": np.ascontiguousarray(z1s.T),
        })

    res = run_bass_kernel_spmd(nc, in_maps, list(range(N_CORES)))

    lv = np.empty((N_PATHS, N_STEPS), np.float32)
    sp = np.empty((N_PATHS, N_STEPS), np.float32)
    lsq = np.empty((N_PATHS,), np.float32)
    for c in range(N_CORES):
        sl = slice(c * P_LOCAL, (c + 1) * P_LOCAL)
        lv[sl] = res.results[c]["lv"]
        sp[sl] = res.results[c]["sp"]
        lsq[sl] = res.results[c]["lsq"]
    return lv, sp, lsq
